# revision 8
# baseline (speedup 1.0000x reference)
"""Trainium2 Bass kernel for nn_CustomModel_21019569946955 (pendulum Lyapunov loss).

Data-parallel over 8 NeuronCores (B/8 = 8192 samples each, weights replicated).
Single fused pass per 256-sample chunk, feature-major activations:

  fwd:  h1 = tanh(W1^T x);  pre2 = W2^T h1;  h2 = tanh(pre2);  [y_pred, V] = W3^T h2
  ode:  v = invert_pend(x, y_pred)            (partition-0 scalar math)
  bwd:  u = W1^T v;  t = (1-h1^2) .* u;  s2 = W2^T t
        Vdot = sum_f (1-h2_f^2) w3_f * s2_f   (per-tile mult + ones-matmul reduce)

The two 2048x2048 W2 contractions (fwd + bwd) dominate. They run on the PE in
fp8 E4M3 DoubleRow mode at 0.5 cycles/row with a 3-product hi/lo split
(measured 2.5e-3 rel err end-to-end in full emulation):

  W2 ~ (W' + Wlo)/64 with W' = q8(64 W2), Wlo = q8(64 W2 - W')   (host-prepped)
  A  ~ A_hi + A_lo   with A_hi = q8(A),   A_lo = q8(A - A_hi)    (on device)
  64 * W2^T A ~ (W'+Wlo)^T A_hi + W'^T A_lo                      (lo*lo dropped)

Each 128-contraction slice-pair (k,k+1) is one DoubleRow matmul, so a full
2048-contraction costs 24 DR matmuls (8 HH + 8 LH + 8 HL) = 12 cycles/row
instead of fp32r's 16. The /64 folds into the activation scale at evacuation.

The backward reuses the SAME stationary fp8 tiles as the forward (Vdot is
computed as (t @ W2) . ((1-h2^2) w3) rather than materializing dV/dx), so
there is no W2 transpose, no DRAM spill, no h1 recompute. A 1-chunk software
pipeline interleaves, per chunk i on the PE:
    [u(i-1) | fwd-DR(i) | pre1(i+1) | yv(i)] then [bwd-DR(i-1) | Vdot(i-1)]
with tanh/quantization on ACT+DVE, d2w3 and the per-sample pendulum ODE on the
Pool engine, so every engine stays busy under the PE's schedule.

custom_loss partial sums (msle/mse) are combined on host (data-parallel mean).
"""
import numpy as np
import ml_dtypes
import concourse.bass as bass
import concourse.tile as tile
from concourse import bacc, mybir
from concourse.bass_utils import run_bass_kernel_spmd

F32 = mybir.dt.float32
F32R = mybir.dt.float32r
F8 = mybir.dt.float8e4
BF16 = mybir.dt.bfloat16
AF = mybir.ActivationFunctionType
ALU = mybir.AluOpType
DR = mybir.MatmulPerfMode.DoubleRow

# problem constants (hardcoded from the reference)
G = 9.8
L, I_, MB, MC, AT, AR = 0.3, 2.0, 1.0, 3.0, 0.2, 0.2
C1 = L * MB            # 0.3
C2 = I_ + L * L * MB   # 2.09
C3 = MB + MC           # 4.0
PEN = 10000.0
ALPHA = 0.1
EPS = 1e-7
C1SQ = C1 * C1
C2C3 = C2 * C3
SW = 64.0              # fp8 weight scale

B, H, D = 65536, 2048, 4
NCORES = 8
BC = B // NCORES        # 8192 samples per core
N = 256                 # chunk size (moving free dim)
CH = BC // N            # 32 chunks
KT = H // 128           # 16 feature tiles
NP = KT // 2            # 8 slice-pairs per 2048-contraction

# fp32 round-to-nearest-int trick + Cody-Waite 2pi for sin/cos range reduction
RC = float(1.5 * 2 ** 23)
INV2PI = float(1.0 / (2.0 * np.pi))
TWOPI_HI = float(np.float32(2.0 * np.pi))
TWOPI_LO = float(2.0 * np.pi - np.float64(np.float32(2.0 * np.pi)))
HALFPI = float(np.pi / 2)

_NC_CACHE = {}


def build():
    nc = bacc.Bacc("TRN2", target_bir_lowering=False, debug=False)

    xtd = nc.declare_dram_parameter("xt", [D, BC], F32, isOutput=False)
    xod = nc.declare_dram_parameter("xob", [3, BC], F32, isOutput=False)
    yd = nc.declare_dram_parameter("y", [BC], F32, isOutput=False)
    W1d = nc.declare_dram_parameter("W1", [D, H], F32, isOutput=False)
    b1d = nc.declare_dram_parameter("b1", [H], F32, isOutput=False)
    W2qd = nc.declare_dram_parameter("W2q", [128, KT, H], F8, isOutput=False)
    W2ld = nc.declare_dram_parameter("W2l", [128, KT, H], F8, isOutput=False)
    b2d = nc.declare_dram_parameter("b2", [H], F32, isOutput=False)
    W3d = nc.declare_dram_parameter("W3", [H, 2], F32, isOutput=False)
    b3d = nc.declare_dram_parameter("b3", [2], F32, isOutput=False)

    loss_out = nc.declare_dram_parameter("loss_pen", [BC], F32, isOutput=True)
    part_out = nc.declare_dram_parameter("partials", [1, 2 * CH], F32,
                                         isOutput=True)

    with tile.TileContext(nc) as tc:
        with tc.tile_pool(name="wpool", bufs=1) as wpool, \
             tc.tile_pool(name="small", bufs=1) as small, \
             tc.tile_pool(name="abuf", bufs=2) as abuf, \
             tc.tile_pool(name="tbuf", bufs=1) as tbuf, \
             tc.tile_pool(name="tmp", bufs=1) as tmp, \
             tc.tile_pool(name="pp_mm1", bufs=3, space="PSUM") as pp_mm1, \
             tc.tile_pool(name="pp_dr", bufs=3, space="PSUM") as pp_dr, \
             tc.tile_pool(name="pp_yv", bufs=1, space="PSUM") as pp_yv, \
             tc.tile_pool(name="pp_vd", bufs=1, space="PSUM") as pp_vd:

            # ---- weights / constants (resident) ----
            w2q = wpool.tile([128, KT, H], F8, tag="w2q", name="w2q")
            nc.sync.dma_start(w2q[:], W2qd[:, :, :])
            w2l = wpool.tile([128, KT, H], F8, tag="w2l", name="w2l")
            nc.sync.dma_start(w2l[:], W2ld[:, :, :])

            w1sb = small.tile([D, KT, 128], F32R, tag="w1", name="w1sb")
            nc.sync.dma_start(w1sb[:],
                              W1d.rearrange("d (k c) -> d k c", c=128).bitcast(F32R))
            w3sb = small.tile([128, KT, 2], F32R, tag="w3", name="w3sb")
            nc.sync.dma_start(w3sb[:],
                              W3d.rearrange("(k p) j -> p k j", p=128).bitcast(F32R))
            b1c = small.tile([128, KT], F32, tag="b1c", name="b1c")
            nc.sync.dma_start(b1c[:], b1d.rearrange("(k p) -> p k", p=128))
            b2c = small.tile([128, KT], F32, tag="b2c", name="b2c")
            nc.sync.dma_start(b2c[:], b2d.rearrange("(k p) -> p k", p=128))
            b3c = small.tile([1, 2], F32, tag="b3c", name="b3c")
            nc.sync.dma_start(b3c[:], b3d.rearrange("(o j) -> o j", o=1))
            w3c1 = small.tile([128, KT, 1], F32, tag="w3c1", name="w3c1")
            nc.sync.dma_start(
                w3c1[:], W3d.rearrange("(k p) j -> p k j", p=128)[:, :, 1:2])
            nw3c1 = small.tile([128, KT, 1], F32, tag="nw3c1", name="nw3c1")
            nc.vector.tensor_scalar_mul(nw3c1[:], w3c1[:], -1.0)
            onesf = small.tile([128, 1], F32, tag="onesf", name="onesf")
            nc.vector.memset(onesf[:], 1.0)
            ones = small.tile([128, 1], F32R, tag="ones", name="ones")
            nc.vector.tensor_copy(ones[:], onesf[:])
            parts = small.tile([1, 2 * CH], F32, tag="parts", name="parts")

            # ---- per-chunk double-buffered SBUF state ----
            def a_hi(i):
                return abuf.tile([128, KT, N], F8, tag="ahi", name="ahi")

            def a_lo(i):
                return abuf.tile([128, KT, N], F8, tag="alo", name="alo")

            def d1t(i, m1):
                return abuf.tile([128, N], BF16, tag=f"d1_{m1}", name="d1t")

            def dwt(i, m2):
                return abuf.tile([128, N], BF16, tag=f"dw_{m2}", name="dwt")

            def xtt(i):
                return abuf.tile([D, N], F32R, tag="xt", name="xtt")

            def xot(i):
                return abuf.tile([1, 3 * N], F32, tag="xo", name="xot")

            def ytt(i):
                return abuf.tile([1, N], F32, tag="yt", name="ytt")

            def vst(i):
                return abuf.tile([D, N], F32R, tag="vs", name="vst")

            # NOTE: the helpers above rotate over bufs=2 slots per call; each
            # is called exactly once per chunk so slot == chunk parity.
            ab_cache = {}

            def cached(fn, i, m=None):
                key = (fn.__name__, i, m)
                if key not in ab_cache:
                    ab_cache[key] = fn(i) if m is None else fn(i, m)
                return ab_cache[key]

            th = tbuf.tile([128, KT, N], F8, tag="th", name="th")
            tl = tbuf.tile([128, KT, N], F8, tag="tl", name="tl")

            # ode workspace: 12 manually-assigned [1, N] slots on partition 0
            ows_t = tbuf.tile([1, 12 * N], F32, tag="ows", name="ows_t")

            def sl(k, w=1):
                return ows_t[:, k * N:(k + w) * N]

            def dma_in(i):
                nc.sync.dma_start(cached(xtt, i)[:],
                                  xtd[:, i * N:(i + 1) * N].bitcast(F32R))
                nc.sync.dma_start(cached(xot, i)[:], xod[:, i * N:(i + 1) * N])
                nc.sync.dma_start(cached(ytt, i)[:], yd[i * N:(i + 1) * N])

            # ---- 24-DR contraction block: psum = 64 * W2^T (hi+lo) ----
            def dr_block(ps, hi, lo, m2):
                ms = slice(m2 * 128, (m2 + 1) * 128)
                for p in range(NP):
                    ks = slice(2 * p, 2 * p + 2)
                    nc.tensor.matmul(ps[:], w2q[:, ks, ms], hi[:, ks, :],
                                     start=(p == 0), stop=False, perf_mode=DR)
                    nc.tensor.matmul(ps[:], w2l[:, ks, ms], hi[:, ks, :],
                                     start=False, stop=False, perf_mode=DR)
                for p in range(NP):
                    ks = slice(2 * p, 2 * p + 2)
                    nc.tensor.matmul(ps[:], w2q[:, ks, ms], lo[:, ks, :],
                                     start=False, stop=(p == NP - 1),
                                     perf_mode=DR)

            # ---- pre1 + tanh + fp8 quant + d1 for (chunk i, tile m1) ----
            def emit_pre1(i, m1):
                psP = pp_mm1.tile([128, N], F32, tag="mm1", name="psP",
                                  padded_shape=[128, 512])
                nc.tensor.matmul(psP[:], w1sb[:, m1], cached(xtt, i)[:],
                                 start=True, stop=True)
                h1t = tmp.tile([128, N], F32, tag="h1t", name="h1t", bufs=3)
                nc.scalar.activation(h1t[:], psP[:], AF.Tanh,
                                     bias=b1c[:, m1:m1 + 1])
                nc.scalar.activation(cached(a_hi, i)[:, m1], h1t[:], AF.Copy)
                nc.vector.tensor_sub(cached(a_lo, i)[:, m1], h1t[:],
                                     cached(a_hi, i)[:, m1])
                sq1 = tmp.tile([128, N], F32, tag="sq1", name="sq1", bufs=2)
                nc.gpsimd.tensor_tensor(sq1[:], h1t[:], h1t[:], ALU.mult)
                nc.gpsimd.tensor_scalar(cached(d1t, i, m1)[:], sq1[:],
                                        -1.0, 1.0, ALU.mult, ALU.add)

            # ---- u-matmul + t hi/lo quant for (chunk i, tile m1) ----
            def emit_ut(i, m1):
                psU = pp_mm1.tile([128, N], F32, tag="mm1", name="psU",
                                  padded_shape=[128, 512])
                nc.tensor.matmul(psU[:], w1sb[:, m1], cached(vst, i)[:],
                                 start=True, stop=True)
                tt = tmp.tile([128, N], F32, tag="tt", name="tt", bufs=2)
                nc.vector.tensor_tensor(tt[:], cached(d1t, i, m1)[:], psU[:],
                                        ALU.mult)
                nc.vector.tensor_copy(th[:, m1], tt[:])
                nc.vector.tensor_sub(tl[:, m1], tt[:], th[:, m1])

            # ---- ode: sin/cos/den terms (needs only xob) ----
            # slots: 0 s, 1 c/p1, 2 rden, 3 cs/x2p, 4 cx4/p2, 5 sx4sq/x4p,
            #        6 csx4sq, 7 yp, 8 vv, 9/10/11 scratch (ypv = 9..11)
            def sin_reduced(src_ap, negate, bias, out):
                w, t_, r = sl(9), sl(10), sl(11)
                nc.gpsimd.tensor_scalar(w[:], src_ap, -1.0 if negate else 1.0,
                                        bias, ALU.mult, ALU.add)
                nc.gpsimd.tensor_scalar(t_[:], w[:], INV2PI, RC,
                                        ALU.mult, ALU.add)
                nc.gpsimd.tensor_scalar(r[:], t_[:], RC, None, ALU.subtract)
                nc.vector.scalar_tensor_tensor(t_[:], r[:], -TWOPI_HI, w[:],
                                               ALU.mult, ALU.add)
                nc.vector.scalar_tensor_tensor(w[:], r[:], -TWOPI_LO, t_[:],
                                               ALU.mult, ALU.add)
                nc.scalar.activation(out[:], w[:], AF.Sin, bias=0.0)

            def emit_ode_pre(i):
                xo = cached(xot, i)
                x3, x4 = xo[:, N:2 * N], xo[:, 2 * N:3 * N]
                sin_reduced(x3, False, 0.0, sl(0))           # s
                sin_reduced(x3, True, HALFPI, sl(1))         # c
                nc.gpsimd.tensor_tensor(sl(9)[:], sl(1)[:], sl(1)[:], ALU.mult)
                nc.gpsimd.tensor_scalar(sl(10)[:], sl(9)[:], -C1SQ, C2C3,
                                        ALU.mult, ALU.add)
                nc.vector.reciprocal(sl(2)[:], sl(10)[:])    # rden
                nc.gpsimd.tensor_tensor(sl(3)[:], sl(1)[:], sl(0)[:], ALU.mult)
                nc.gpsimd.tensor_tensor(sl(9)[:], x4, x4, ALU.mult)  # x4sq
                nc.gpsimd.tensor_tensor(sl(4)[:], sl(1)[:], x4, ALU.mult)
                nc.gpsimd.tensor_tensor(sl(5)[:], sl(0)[:], sl(9)[:], ALU.mult)
                nc.gpsimd.tensor_tensor(sl(6)[:], sl(3)[:], sl(9)[:], ALU.mult)

            # ---- ode tail + v assembly + msle/pen1 (needs y_pred) ----
            def emit_ode_post(i, yvp, pen1):
                xo = cached(xot, i)
                x2, x4 = xo[:, 0:N], xo[:, 2 * N:3 * N]
                yv2 = tmp.tile([2, N], F32, tag="yv2", name="yv2", bufs=2)
                nc.vector.tensor_copy(yv2[:], yvp[:])
                nc.sync.dma_start(sl(9, 2), yv2[:])          # ypv -> slots 9,10
                nc.vector.tensor_scalar(sl(7)[:], sl(9)[:], b3c[0:1, 0:1],
                                        None, ALU.add)       # yp
                nc.vector.tensor_scalar(sl(8)[:], sl(10)[:], b3c[0:1, 1:2],
                                        None, ALU.add)       # vv
                f, cf = sl(11), sl(10)
                nc.vector.scalar_tensor_tensor(f[:], x2, -AT, sl(7)[:],
                                               ALU.mult, ALU.add)
                nc.gpsimd.tensor_tensor(cf[:], sl(1)[:], f[:], ALU.mult)
                p1 = sl(1)  # c dead
                nc.gpsimd.tensor_scalar(p1[:], f[:], C2, None, ALU.mult)
                nc.vector.scalar_tensor_tensor(p1[:], sl(3)[:], G * C1SQ,
                                               p1[:], ALU.mult, ALU.add)
                nc.vector.scalar_tensor_tensor(p1[:], sl(4)[:], -AR * C1,
                                               p1[:], ALU.mult, ALU.add)
                nc.vector.scalar_tensor_tensor(p1[:], sl(5)[:], -C1 * C2,
                                               p1[:], ALU.mult, ALU.add)
                nc.gpsimd.tensor_tensor(sl(3)[:], p1[:], sl(2)[:], ALU.mult)
                p2 = sl(4)  # cx4 dead
                nc.gpsimd.tensor_scalar(p2[:], sl(0)[:], G * C1 * C3,
                                        None, ALU.mult)
                nc.vector.scalar_tensor_tensor(p2[:], cf[:], C1, p2[:],
                                               ALU.mult, ALU.add)
                nc.vector.scalar_tensor_tensor(p2[:], x4, -AR * C3, p2[:],
                                               ALU.mult, ALU.add)
                nc.vector.scalar_tensor_tensor(p2[:], sl(6)[:], -C1SQ, p2[:],
                                               ALU.mult, ALU.add)
                nc.gpsimd.tensor_tensor(sl(5)[:], p2[:], sl(2)[:], ALU.mult)
                vs = cached(vst, i)
                nc.sync.dma_start(vs[0:1, :], x2.bitcast(F32R))
                nc.sync.dma_start(vs[1:2, :], sl(3).bitcast(F32R))         # x2p
                nc.sync.dma_start(vs[2:3, :], x4.bitcast(F32R))
                nc.sync.dma_start(vs[3:4, :], sl(5).bitcast(F32R))         # x4p
                # msle / mse partial sums
                yt = cached(ytt, i)
                nc.vector.tensor_scalar(sl(9)[:], sl(7)[:], EPS, None, ALU.max)
                nc.scalar.activation(sl(10)[:], sl(9)[:], AF.Ln, bias=1.0)
                nc.vector.tensor_scalar(sl(11)[:], yt[:], EPS, None, ALU.max)
                nc.scalar.activation(sl(9)[:], sl(11)[:], AF.Ln, bias=1.0)
                nc.vector.tensor_sub(sl(11)[:], sl(10)[:], sl(9)[:])
                nc.scalar.activation(sl(9)[:], sl(11)[:], AF.Square,
                                     accum_out=parts[0:1, i:i + 1])
                nc.vector.tensor_sub(sl(10)[:], yt[:], sl(7)[:])
                nc.scalar.activation(sl(9)[:], sl(10)[:], AF.Square,
                                     accum_out=parts[0:1, CH + i:CH + i + 1])
                nc.vector.tensor_scalar(pen1[:], sl(8)[:], 0.0, -PEN,
                                        ALU.min, ALU.mult)

            # ---- bwd DR + Vdot reduce + loss tail for chunk j ----
            def emit_bwd(j, pen1):
                vdp = pp_vd.tile([1, N], F32, tag="vd", name="vdp",
                                 padded_shape=[1, 512])
                vts = [None] * KT
                for m2 in range(KT):
                    psB = pp_dr.tile([128, N], F32, tag="dr", name="psB",
                                     padded_shape=[128, 512])
                    dr_block(psB, th, tl, m2)
                    vt = tmp.tile([128, N], F32R, tag="vt", name="vt", bufs=3)
                    nc.vector.tensor_tensor(vt[:], cached(dwt, j, m2)[:],
                                            psB[:], ALU.mult)
                    vts[m2] = vt
                    if m2 >= 1:
                        nc.tensor.matmul(vdp[:], ones[:], vts[m2 - 1][:],
                                         start=(m2 == 1), stop=False)
                nc.tensor.matmul(vdp[:], ones[:], vts[KT - 1][:],
                                 start=False, stop=True)
                penT = tmp.tile([1, N], F32, tag="penT", name="penT", bufs=2)
                nc.vector.tensor_scalar(penT[:], vdp[:], 0.0, PEN / SW,
                                        ALU.max, ALU.mult)
                nc.vector.tensor_add(penT[:], penT[:], pen1[:])
                nc.sync.dma_start(
                    loss_out[j * N:(j + 1) * N].rearrange("(o n) -> o n", o=1),
                    penT[:])

            # ================= pipeline =================
            pen1s = [None] * CH
            dma_in(0)
            for m1 in range(KT):
                emit_pre1(0, m1)

            for i in range(CH):
                if i + 1 < CH:
                    dma_in(i + 1)
                yvp = pp_yv.tile([2, N], F32, tag="yv", name="yvp",
                                 padded_shape=[2, 512])
                h2ts = [None] * KT
                for m2 in range(KT):
                    if i >= 1:
                        emit_ut(i - 1, m2)
                    psD = pp_dr.tile([128, N], F32, tag="dr", name="psD",
                                     padded_shape=[128, 512])
                    dr_block(psD, cached(a_hi, i), cached(a_lo, i), m2)
                    if i + 1 < CH:
                        emit_pre1(i + 1, m2)
                    h2t = tmp.tile([128, N], F32R, tag="h2t", name="h2t",
                                   bufs=3)
                    nc.scalar.activation(h2t[:], psD[:], AF.Tanh,
                                         bias=b2c[:, m2:m2 + 1],
                                         scale=1.0 / SW)
                    h2ts[m2] = h2t
                    sqh = tmp.tile([128, N], F32, tag="sqh", name="sqh",
                                   bufs=2)
                    nc.gpsimd.tensor_tensor(sqh[:], h2t[:], h2t[:], ALU.mult)
                    nc.gpsimd.tensor_scalar(cached(dwt, i, m2)[:], sqh[:],
                                            nw3c1[:, m2], w3c1[:, m2],
                                            ALU.mult, ALU.add)
                    if m2 >= 2:
                        nc.tensor.matmul(yvp[:], w3sb[:, m2 - 2],
                                         h2ts[m2 - 2][:],
                                         start=(m2 == 2), stop=False)
                    if m2 == 8:
                        emit_ode_pre(i)
                for m2 in (KT - 2, KT - 1):
                    nc.tensor.matmul(yvp[:], w3sb[:, m2], h2ts[m2][:],
                                     start=False, stop=(m2 == KT - 1))

                pen1s[i] = tmp.tile([1, N], F32, tag="pen1", name="pen1",
                                    bufs=2)
                emit_ode_post(i, yvp, pen1s[i])
                if i >= 1:
                    emit_bwd(i - 1, pen1s[i - 1])

            for m2 in range(KT):
                emit_ut(CH - 1, m2)
            emit_bwd(CH - 1, pen1s[CH - 1])

            nc.sync.dma_start(part_out[:, :], parts[:])

    nc.compile()
    return nc


def _prep_inputs(inputs):
    x = np.ascontiguousarray(inputs["x"], dtype=np.float32)
    y = np.ascontiguousarray(inputs["y"], dtype=np.float32)
    W1 = np.ascontiguousarray(inputs["W1"], dtype=np.float32)
    b1 = np.ascontiguousarray(inputs["b1"], dtype=np.float32)
    W2 = np.ascontiguousarray(inputs["W2"], dtype=np.float32)
    b2 = np.ascontiguousarray(inputs["b2"], dtype=np.float32)
    W3 = np.ascontiguousarray(inputs["W3"], dtype=np.float32)
    b3 = np.ascontiguousarray(inputs["b3"], dtype=np.float32)

    f8 = ml_dtypes.float8_e4m3
    W2s = (SW * W2).astype(np.float32)
    W2q = W2s.astype(f8)
    W2l = (W2s - W2q.astype(np.float32)).astype(f8)
    # [H, H] -> [128(within-slice), KT(slice), H(out feature)]
    W2q = np.ascontiguousarray(W2q.reshape(KT, 128, H).transpose(1, 0, 2))
    W2l = np.ascontiguousarray(W2l.reshape(KT, 128, H).transpose(1, 0, 2))

    xt_full = np.ascontiguousarray(x.T)              # [4, B]
    xob_full = np.ascontiguousarray(x[:, 1:4].T)     # [3, B] rows x2,x3,x4
    return y, xt_full, xob_full, W1, b1, W2q, W2l, b2, W3, b3


def make_in_maps(inputs):
    y, xt_full, xob_full, W1, b1, W2q, W2l, b2, W3, b3 = _prep_inputs(inputs)
    in_maps = []
    for cid in range(NCORES):
        sl_ = slice(cid * BC, (cid + 1) * BC)
        in_maps.append({
            "xt": np.ascontiguousarray(xt_full[:, sl_]),
            "xob": np.ascontiguousarray(xob_full[:, sl_]),
            "y": np.ascontiguousarray(y[sl_]),
            "W1": W1, "b1": b1, "W2q": W2q, "W2l": W2l,
            "b2": b2, "W3": W3, "b3": b3,
        })
    return in_maps


def kernel(**inputs):
    if "nc" not in _NC_CACHE:
        _NC_CACHE["nc"] = build()
    nc = _NC_CACHE["nc"]

    in_maps = make_in_maps(inputs)
    res = run_bass_kernel_spmd(nc, in_maps, list(range(NCORES)))

    loss = np.concatenate([res.results[c]["loss_pen"] for c in range(NCORES)])
    parts = np.stack([res.results[c]["partials"] for c in range(NCORES)])
    sums = parts.astype(np.float64).reshape(NCORES, 2, CH).sum(axis=(0, 2))
    scalar = ALPHA * sums[0] / B + (1.0 - ALPHA) * sums[1] / B
    return (loss + np.float32(scalar)).astype(np.float32)
